# revision 20
# baseline (speedup 1.0000x reference)
"""Trainium2 Bass kernel for nn_AndLayer (permutation-based AND layer).

Math (see reference):
    tk = tanh(kernel)                 # [448, C=128]
    q  = 1 - tk^2
    For each batch b and permutation k=(o0,o1) of 8 objects (K=56 perms):
        in_vec[448] = [nullary(64) | unary[o0](128) | unary[o1](128)
                       | binary[o0,o1'](64) | binary[o1,o0'](64)]
        conj[b,k,c] = min_i (in_vec[i]*tk[i,c] + q[i,c])
        out[b,c]    = max_k conj[b,k,c]

Decomposition (exact):
    nmin[b,c]     = min over nullary 64 rows        (shared by all k)
    umin_v[b,o,c] = min over unary   128 rows       (16 combos, not 112)
    bmin[b,k,c]   = min over binary v0+v1 128 rows  (per k)
    conj = min(nmin, umin0[o0], umin1[o1], bmin[k]); out = max_k conj

Device strategy (per core, data-parallel over B: 4 batches/core):
    One matmul per 64-pred half-tile computes in*tk + q directly: the
    stationary stacks [tk_half ; q_half] (K=128) and the moving operand
    stacks [diag(in_half) ; I64] (prepared host-side as pure data
    rearrangement). Output lands in PSUM transposed ([c, pred]) so the
    min-reduce is a free-axis reduce. The PSUM drain is split between
    the Vector engine (direct tensor_reduce) and the Scalar engine
    (ACT copy to bf16 SBUF, then a 2x-mode tensor_tensor min tree on
    Vector), since each engine drains PSUM at ~1 elem/cycle/lane.
"""

import itertools
import os
import sys

import numpy as np

for _p in ("/opt/trn_rl_repo", "/root/.axon_site/_ro/trn_rl_repo"):
    if os.path.isdir(_p) and _p not in sys.path:
        sys.path.insert(0, _p)

import concourse.bass as bass  # noqa: E402
import concourse.bacc as bacc  # noqa: E402
import concourse.mybir as mybir  # noqa: E402
import concourse.tile as tile  # noqa: E402
from concourse.bass_utils import run_bass_kernel_spmd  # noqa: E402

import ml_dtypes  # noqa: E402

BF16 = ml_dtypes.bfloat16

# Problem constants (hardcoded per spec)
B, N, V = 32, 8, 2
P0, P1, P2, C = 64, 128, 64, 128
K = 56  # permutations of 2 from 8
NCORES = 8
BL = B // NCORES  # 4 batches per core
NBT = BL * K  # binary tiles per core = 224
NWAVE = NBT // 16  # 14 binary waves of 16 tiles

# Waves drained directly by vector tensor_reduce (rest: scalar copy + tree).
# Spread through the schedule so vector fills the gaps while scalar drains.
DIRECT_SET = {0, 1, 5, 9, 13}
DIRECT_UNARY = False

F32 = mybir.dt.float32
BF16_T = mybir.dt.bfloat16
MIN_OP = mybir.AluOpType.min


def _perm_tables():
    perm_idxs = np.array(list(itertools.permutations(range(N), V)))  # [56, 2]
    k_of = {tuple(p): i for i, p in enumerate(perm_idxs)}
    rev = np.array([k_of[(p[1], p[0])] for p in perm_idxs])  # index of (o1,o0)
    return perm_idxs, rev


PERM_IDXS, REV = _perm_tables()


def build_graph():
    nc = bacc.Bacc("TRN2", debug=False)

    abin = nc.declare_dram_parameter("abin", [128, NBT * 128], BF16_T, isOutput=False)
    aun = nc.declare_dram_parameter("aun", [128, BL * N * 128], BF16_T, isOutput=False)
    anul = nc.declare_dram_parameter("anul", [128, BL * 64], BF16_T, isOutput=False)
    kern = nc.declare_dram_parameter("kern", [448, 128], F32, isOutput=False)
    out_d = nc.declare_dram_parameter("out", [128, BL], F32, isOutput=True)

    with tile.TileContext(nc) as tc:
        with (
            tc.tile_pool(name="const", bufs=1) as const,
            tc.tile_pool(name="work", bufs=2) as work,
            tc.tile_pool(name="tree", bufs=3) as treep,
            tc.tile_pool(name="psum", bufs=2, space="PSUM") as psum_pool,
        ):
            # ---- stationaries: ST[j] = [tanh(kern[64j:64j+64]) ; 1-tanh^2].
            # All 7 kernel chunks land with 2 DMAs (rows replicated into both
            # partition halves); binary stationaries (j=5,6) built first so
            # the matmul pipeline can start while the rest are prepared. ----
            raw_all = const.tile([128, 7 * 128], F32, tag="rawall")
            ksrc = kern[:].rearrange("(j p) c -> p j c", p=64)
            nc.sync.dma_start(raw_all[0:64, :].rearrange("p (j c) -> p j c", c=128), ksrc)
            nc.sync.dma_start(raw_all[64:128, :].rearrange("p (j c) -> p j c", c=128), ksrc)
            st_all = const.tile([128, 7 * 128], BF16_T, tag="stall")
            sq = work.tile([64, 7 * 128], F32, tag="sqall")
            qf = work.tile([64, 7 * 128], F32, tag="qfall")
            # binary stationaries (cols 640:896) first so matmuls start early
            for lo, hi in ((640, 896), (0, 640)):
                nc.scalar.activation(
                    st_all[:, lo:hi],
                    raw_all[:, lo:hi],
                    mybir.ActivationFunctionType.Tanh,
                )
                nc.scalar.activation(
                    sq[:, lo:hi],
                    st_all[64:128, lo:hi],
                    mybir.ActivationFunctionType.Square,
                )
                nc.vector.tensor_scalar(
                    qf[:, lo:hi], sq[:, lo:hi], -1.0, 1.0,
                    mybir.AluOpType.mult, mybir.AluOpType.add,
                )
                nc.vector.tensor_copy(st_all[64:128, lo:hi], qf[:, lo:hi])
            sts = [st_all[:, j * 128 : (j + 1) * 128] for j in range(7)]
            st_n, st_u0a, st_u0b, st_u1a, st_u1b, st_ba, st_bb = sts

            # ---- atlases: one SBUF tile per wave so compute starts as
            # soon as its slice lands (tile-granular deps) ----
            abin_w = []
            for wp in range((NWAVE + 1) // 2):
                n = min(2, NWAVE - wp * 2)
                t = const.tile([128, 2048 * n], BF16_T, tag=f"abw{wp}")
                nc.sync.dma_start(
                    t[:], abin[:, wp * 4096 : wp * 4096 + 2048 * n]
                )
                for i in range(n):
                    abin_w.append(t[:, i * 2048 : (i + 1) * 2048])
            aun_all = const.tile([128, BL * N * 128], BF16_T, tag="aunall")
            nc.sync.dma_start(aun_all[:], aun[:])
            aun_w = [
                aun_all[:, g * 1024 : (g + 1) * 1024] for g in range(BL * N // 8)
            ]
            anul_s = const.tile([128, BL * 64], BF16_T, tag="anul")
            nc.sync.dma_start(anul_s[:], anul[:])

            # ---- result accumulators ([c, tiles], bf16) ----
            bmin_all = const.tile([128, NBT], BF16_T, tag="bmin")
            um_all = const.tile([128, BL * 16], BF16_T, tag="um")  # [um0|um1] per b
            nm_all = const.tile([128, BL], BF16_T, tag="nm")

            def drain_tree(ps, out_ap):
                """Drain one 2048-wide psum wave ([Ag|Bg|Ag'|Bg'] layout) into
                16 per-tile mins. Scalar copies psum -> bf16 SBUF; vector does
                a 2x-mode min tree then a short reduce -> out_ap [128, 16]."""
                scr = treep.tile([128, 2048], BF16_T, tag="scr")
                nc.scalar.activation(
                    scr[:], ps[:, 0:2048], mybir.ActivationFunctionType.Copy
                )
                t0 = treep.tile([128, 1024], BF16_T, tag="t0")
                nc.vector.tensor_tensor(
                    t0[:], scr[:, 0:1024], scr[:, 1024:2048], MIN_OP,
                )
                hl = t0[:].rearrange("p (t c f) -> p c t f", c=2, f=32)
                t1 = treep.tile([128, 512], BF16_T, tag="t1")
                nc.vector.tensor_tensor(
                    t1[:].rearrange("p (t f) -> p t f", f=32),
                    hl[:, 0], hl[:, 1], MIN_OP,
                )
                nc.vector.tensor_reduce(
                    out_ap,
                    t1[:].rearrange("p (t f) -> p t f", f=32),
                    mybir.AxisListType.X,
                    MIN_OP,
                )

            def drain_tree_u(ps, out_ap):
                """Unary wave drain: psum layout [u0A x8|u0B x8|u1A x8|u1B x8];
                min A vs B within each half -> [um0 (8) | um1 (8)]."""
                scr = treep.tile([128, 2048], BF16_T, tag="scr")
                nc.scalar.activation(
                    scr[:], ps[:, 0:2048], mybir.ActivationFunctionType.Copy
                )
                ab = scr[:].rearrange("p (h c i f) -> p c h i f", h=2, c=2, f=64)
                t0 = treep.tile([128, 1024], BF16_T, tag="t0")
                nc.vector.tensor_tensor(
                    t0[:].rearrange("p (h i f) -> p h i f", h=2, f=64),
                    ab[:, 0], ab[:, 1], MIN_OP,
                )
                hl = t0[:].rearrange("p (t c f) -> p c t f", c=2, f=32)
                t1 = treep.tile([128, 512], BF16_T, tag="t1")
                nc.vector.tensor_tensor(
                    t1[:].rearrange("p (t f) -> p t f", f=32),
                    hl[:, 0], hl[:, 1], MIN_OP,
                )
                nc.vector.tensor_reduce(
                    out_ap,
                    t1[:].rearrange("p (t f) -> p t f", f=32),
                    mybir.AxisListType.X,
                    MIN_OP,
                )

            # ---- binary waves: 16 tiles each ----
            # atlas/psum layout per wave: [A-blocks x16 | B-blocks x16].
            for w in range(NWAVE):
                ps = psum_pool.tile([128, 2048], F32, tag="pswave")
                src = abin_w[w]
                nc.tensor.matmul(
                    ps[:, 0:512], st_ba, src[:, 0:512], start=True, stop=True
                )
                nc.tensor.matmul(
                    ps[:, 512:1024], st_ba, src[:, 512:1024], start=True, stop=True
                )
                nc.tensor.matmul(
                    ps[:, 1024:1536], st_bb, src[:, 1024:1536], start=True, stop=True
                )
                nc.tensor.matmul(
                    ps[:, 1536:2048], st_bb, src[:, 1536:2048], start=True, stop=True
                )
                if w in DIRECT_SET:
                    nc.vector.tensor_reduce(
                        bmin_all[:, w * 16 : (w + 1) * 16],
                        ps[:].rearrange("p (c i f) -> p i c f", c=2, f=64),
                        mybir.AxisListType.XY,
                        MIN_OP,
                    )
                else:
                    drain_tree(ps, bmin_all[:, w * 16 : (w + 1) * 16])

            # ---- unary waves: 8 (b,o) tiles each, 4 waves ----
            # psum: [u0A | u0B | u1A | u1B]; u0 = banks 0-1, u1 = banks 2-3.
            for g in range(BL * N // 8):
                ps = psum_pool.tile([128, 2048], F32, tag="pswave")
                src = aun_w[g]
                nc.tensor.matmul(
                    ps[:, 0:512], st_u0a, src[:, 0:512], start=True, stop=True
                )
                nc.tensor.matmul(
                    ps[:, 512:1024], st_u0b, src[:, 512:1024], start=True, stop=True
                )
                nc.tensor.matmul(
                    ps[:, 1024:1536], st_u1a, src[:, 0:512], start=True, stop=True
                )
                nc.tensor.matmul(
                    ps[:, 1536:2048], st_u1b, src[:, 512:1024], start=True, stop=True
                )
                if DIRECT_UNARY:
                    for half in range(2):
                        nc.vector.tensor_reduce(
                            um_all[:, g * 16 + half * 8 : g * 16 + half * 8 + 8],
                            ps[:, half * 1024 : (half + 1) * 1024].rearrange(
                                "p (c i f) -> p i c f", c=2, f=64
                            ),
                            mybir.AxisListType.XY,
                            MIN_OP,
                        )
                else:
                    drain_tree_u(ps, um_all[:, g * 16 : (g + 1) * 16])

            # ---- nullary: one matmul for all 4 batches ----
            psn = psum_pool.tile([128, 256], F32, tag="pswave")
            nc.tensor.matmul(psn[:], st_n, anul_s[:], start=True, stop=True)
            nc.vector.tensor_reduce(
                nm_all[:],
                psn[:].rearrange("p (t f) -> p t f", f=64),
                mybir.AxisListType.X,
                MIN_OP,
            )

            # ---- combine (batched over the 4 batches) ----
            grid = work.tile([128, BL * 64], BF16_T, tag="grid")
            bm3 = bmin_all[:].rearrange("p (k b) -> p b k", b=BL)
            g3 = grid[:].rearrange("p (b f) -> p b f", b=BL)
            for o0 in range(8):
                if o0 > 0:
                    nc.vector.tensor_copy(
                        g3[:, :, o0 * 8 : o0 * 8 + o0],
                        bm3[:, :, o0 * 7 : o0 * 7 + o0],
                    )
                if o0 < 7:
                    nc.vector.tensor_copy(
                        g3[:, :, o0 * 8 + o0 + 1 : o0 * 8 + 8],
                        bm3[:, :, o0 * 7 + o0 : o0 * 7 + 7],
                    )
            nc.vector.memset(g3[:, :, 0:64:9], -3.0e38)
            # min with unary v0 term: value indexed by (b, o0)
            g4 = grid[:].rearrange("p (b i j) -> p b i j", b=BL, i=8)
            um3 = um_all[:].rearrange("p (b x) -> p b x", b=BL)
            u0b = um3[:, :, 0:8].unsqueeze(3).to_broadcast((128, BL, 8, 8))
            nc.vector.tensor_tensor(g4, g4, u0b, MIN_OP)
            # min with unary v1 term: value indexed by (b, o1)
            u1b = um3[:, :, 8:16].unsqueeze(2).to_broadcast((128, BL, 8, 8))
            nc.vector.tensor_tensor(g4, g4, u1b, MIN_OP)
            # min with nullary term: value indexed by b
            nmb = nm_all[:].unsqueeze(2).to_broadcast((128, BL, 64))
            nc.vector.tensor_tensor(g3, g3, nmb, MIN_OP)
            # max over the 56 valid permutations
            outcb = work.tile([128, BL], F32, tag="outcb")
            nc.vector.tensor_reduce(
                outcb[:], g3, mybir.AxisListType.X, mybir.AluOpType.max
            )

            # write out c-major; host transposes to [b, c]
            nc.sync.dma_start(out_d[:], outcb[:])

    nc.compile()
    return nc


def _diag_atlas(scales_a, scales_b, grp):
    """Pack [diag(sA);I64] / [diag(sB);I64] blocks in groups of `grp` tiles:
    per group g: cols [g*grp*128 : +grp*64] = A-blocks, then B-blocks.
    scales_*: [T, 64]. Returns [128, T*128] bf16."""
    T = scales_a.shape[0]
    atlas = np.zeros((128, T * 128), dtype=BF16)
    t = np.arange(T)
    j = np.arange(64)
    base_a = ((t // grp) * grp * 128 + (t % grp) * 64)[:, None] + j[None, :]
    base_b = base_a + grp * 64
    atlas[j[None, :], base_a] = scales_a.astype(BF16)
    atlas[64 + j[None, :], base_a] = 1.0
    atlas[j[None, :], base_b] = scales_b.astype(BF16)
    atlas[64 + j[None, :], base_b] = 1.0
    return atlas


def make_core_inputs(nul, una, binf, ker):
    """Per-core in_map. nul [4,64], una [4,8,128], binf [4,56,64] f32."""
    bl = nul.shape[0]
    # binary: tile t=b*56+k: A-scale = binf[b,k] (v0), B-scale = binf[b,rev k]
    # k-major tile order (t = k*BL + b): combine copy inputs finish early
    sv0 = binf.transpose(1, 0, 2).reshape(bl * K, 64)
    sv1 = binf[:, REV].transpose(1, 0, 2).reshape(bl * K, 64)
    abin = _diag_atlas(sv0, sv1, 16)
    # unary: tile t=b*8+o: A = una[b,o,:64], B = una[b,o,64:]
    su = una.reshape(bl * N, 128)
    aun = _diag_atlas(su[:, :64], su[:, 64:], 8)
    # nullary: block b: [diag(nul[b]) ; I64]
    anul = np.zeros((128, bl * 64), dtype=BF16)
    j = np.arange(64)
    base = (np.arange(bl) * 64)[:, None] + j[None, :]
    anul[j[None, :], base] = nul.astype(BF16)
    anul[64 + j[None, :], base] = 1.0

    return {
        "abin": abin,
        "aun": aun,
        "anul": anul,
        "kern": np.ascontiguousarray(ker.astype(np.float32)),
    }


LAST_RESULTS = None
_GRAPH_CACHE = {}


def get_graph():
    if "nc" not in _GRAPH_CACHE:
        _GRAPH_CACHE["nc"] = build_graph()
    return _GRAPH_CACHE["nc"]


def kernel(nullary_preds, unary_preds, binary_preds, kernel):
    nul = np.asarray(nullary_preds, dtype=np.float32)
    una = np.asarray(unary_preds, dtype=np.float32)
    binf = np.asarray(binary_preds, dtype=np.float32).reshape(B, K, P2)
    ker = np.asarray(kernel, dtype=np.float32)

    nc = get_graph()
    in_maps = []
    for core in range(NCORES):
        bs = slice(core * BL, (core + 1) * BL)
        in_maps.append(make_core_inputs(nul[bs], una[bs], binf[bs], ker))
    res = run_bass_kernel_spmd(nc, in_maps, core_ids=list(range(NCORES)))
    global LAST_RESULTS
    LAST_RESULTS = res
    out = np.concatenate(
        [np.asarray(res.results[i]["out"]).T for i in range(NCORES)], 0
    )
    return out.astype(np.float32)
